# revision 4
# baseline (speedup 1.0000x reference)
"""Trainium2 Bass kernel for the masked-FFT CG data-consistency problem.

Math: the reference runs 10 CG iterations on (A^H A + lam I) x = atbT + lam z
where A^H A = ifft2(mask * fft2(.)) is DIAGONAL in the Fourier basis with
eigenvalue d = mask + lam per mode.  CG therefore collapses to a spectral
filter chi(d):  out = ifft2(chi * fft2(rhs)).  The exact CG filter depends on
per-mode weights w_j = sum_b |rhs_hat|^2, but with 262k modes densely covering
[lam, 1+lam] the filter is insensitive to the weights: the uniform-w filter
(computable on host from mask alone) is within 2.2e-5 of the true-w one —
far inside tolerance.  So chi is host-precomputed and the whole problem is a
single fused device kernel: rhs = atbT + lam z; H = FFT2(rhs); G = chi*H;
out = IFFT2(G).  No inter-kernel HBM round trip, no w reduction.

Device work = batched 512x512 FFT2 / IFFT2 as radix-2 DFT matmuls
batch-sharded 2 slices/core over 8 cores.  Stationary data blocks,
moving DFT consts, and intermediate planes are all BF16 (halves LDWEIGHTS
time and SBUF/DMA for consts; PSUM accumulation stays f32; measured rel err
~4.5e-3 vs the 2e-2 gate).  Inputs/outputs stay f32.

Each FFT2 is two matmul passes with the DATA blocks stationary and the DFT
matrices moving: pass(X) = (F @ X).T, so pass(pass(X)) = F X F = fft2(X), no
transposes.  Radix-2 splits rows even/odd (K=256 per part, twiddles folded
into the odd-part moving matrices); moving consts pack [re|im] halves so one
matmul fills [E_re|E_im] of a PSUM bank; E +/- T recombines on the vector
engine during eviction (T staged through SBUF by the scalar engine - DVE
cannot read two PSUM operands).  Rows use a parity-grouped layout
sigma(jt, p) = 2*((jt % 2)*128 + p) + jt//2, preserved across passes by
selecting stride-2 column blocks, so no partition permutes are needed.
bf16 dummy matmuls warm the PE HAM clock while input DMAs stream.
"""

import numpy as np

LAM = 0.05
CG_ITER = 10
B_FULL, H, W = 16, 512, 512
JT, P = 4, 128
N_CORES = 8

_cache = {}


def _perm_rows():
    idx = np.zeros(512, np.int64)
    for jt in range(4):
        for p in range(128):
            idx[jt * 128 + p] = 2 * ((jt % 2) * 128 + p) + jt // 2
    return idx


def _make_consts(conj):
    import ml_dtypes

    m = np.arange(256)
    k1 = np.arange(256)
    we = np.exp(-2j * np.pi * np.outer(m, k1) / 256)
    wt = we * np.exp(-2j * np.pi * k1 / 512)[None, :]

    def comp(a, b):
        M = np.concatenate([a, b], axis=1)
        return np.ascontiguousarray(
            M.astype(np.float32).astype(ml_dtypes.bfloat16).reshape(2, 128, 512))

    if not conj:
        return (comp(we.real, we.imag), comp(-we.imag, we.real),
                comp(wt.real, wt.imag), comp(-wt.imag, wt.real))
    return (comp(we.real, -we.imag), comp(we.imag, we.real),
            comp(wt.real, -wt.imag), comp(wt.imag, wt.real))


def _collapsed_cg(d, w, iters=CG_ITER, tol=1e-10):
    d = d.astype(np.float64).ravel()
    w = w.astype(np.float64).ravel()
    q = np.ones_like(d)
    s = np.ones_like(d)
    chi = np.zeros_like(d)
    rTr = (q * q * w).sum()
    for _ in range(iters):
        if abs(rTr) <= tol:
            break
        denom = (d * s * s * w).sum()
        alpha = rTr / denom
        chi = chi + alpha * s
        q = q - alpha * d * s
        rTr_new = (q * q * w).sum()
        beta = rTr_new / rTr
        s = q + beta * s
        rTr = rTr_new
    return chi.reshape(512, 512)


def _build_kernel():
    import concourse.mybir as mybir
    import concourse.tile as tile
    from concourse import bacc

    dt_bf = mybir.dt.bfloat16

    def load_consts(nc, cpool, aps, names):
        tiles = []
        for name, ap in zip(names, aps):
            t = cpool.tile([P, 2, 512], dt_bf, tag=name, name=name)
            nc.sync.dma_start(t[:], ap.rearrange("kt p c -> p kt c"))
            tiles.append(t)
        return tiles

    def warmup(nc, cpool, psp, n=28):
        wb = cpool.tile([P, 128], mybir.dt.bfloat16, tag="wb", name="wb")
        mb = cpool.tile([P, 512], mybir.dt.bfloat16, tag="mb", name="mb")
        nc.vector.memset(wb[:], 0.0)
        nc.vector.memset(mb[:], 0.0)
        for i in range(n):
            pw = psp.tile([P, 512], mybir.dt.float32, tag="pse", name=f"pw{i}")
            nc.tensor.matmul(pw[:], wb[:], mb[:], start=True, stop=True)

    def dft_pass(nc, psp, dpool, stat, G3, emit, qs=(0, 1, 2, 3), tsb_dt=None):
        a1, a2, t1, t2 = G3
        for q in qs:
            ps_e = psp.tile([P, 512], mybir.dt.float32, tag="pse", name=f"pse{q}")
            ps_t = psp.tile([P, 512], mybir.dt.float32, tag="pst", name=f"pst{q}")
            for part, jts, m1, m2 in (("E", (0, 1), a1, a2), ("T", (2, 3), t1, t2)):
                ps = ps_e if part == "E" else ps_t
                for kt in range(2):
                    nc.tensor.matmul(ps[:], stat(jts[kt], q, 0), m1[:, kt, :],
                                     start=(kt == 0), stop=False)
                    nc.tensor.matmul(ps[:], stat(jts[kt], q, 1), m2[:, kt, :],
                                     start=False, stop=(kt == 1))
            t_sb = dpool.tile([P, 512], tsb_dt, tag="tsb", name=f"tsb{q}")
            nc.scalar.copy(t_sb[:], ps_t[:])
            emit(q, ps_e, t_sb)

    def comb(nc, plane, q, ps_e, t_sb):
        e2 = ps_e[:].rearrange("p (k c) -> p k c", k=2)
        t2 = t_sb[:].rearrange("p (k c) -> p k c", k=2)
        nc.vector.tensor_add(plane[:, q, :, 0:256], e2, t2)
        nc.vector.tensor_sub(plane[:, q, :, 256:512], e2, t2)

    def build():
        nc = bacc.Bacc("TRN2", target_bir_lowering=False, debug=False,
                       num_devices=N_CORES)
        zs = nc.dram_tensor("zs", [2, H, W, 2], mybir.dt.float32,
                            kind="ExternalInput").ap()
        as_ = nc.dram_tensor("as_", [2, H, W, 2], mybir.dt.float32,
                             kind="ExternalInput").ap()
        fwd_names = ["a1", "a2", "t1", "t2"]
        con_names = ["c1", "c2", "c3", "c4"]
        gaps_f = [nc.dram_tensor(n, [2, P, 512], dt_bf, kind="ExternalInput").ap()
                  for n in fwd_names]
        gaps_c = [nc.dram_tensor(n, [2, P, 512], dt_bf, kind="ExternalInput").ap()
                  for n in con_names]
        chi = nc.dram_tensor("chi", [JT, P, W], dt_bf,
                             kind="ExternalInput").ap()
        out = nc.dram_tensor("out", [2, H, W, 2], mybir.dt.float32,
                             kind="ExternalOutput").ap()

        with tile.TileContext(nc) as tc:
            with (
                tc.tile_pool(name="const", bufs=1) as cpool,
                tc.tile_pool(name="data", bufs=2) as dpool,
                tc.tile_pool(name="ps", bufs=4, space="PSUM") as psp,
            ):
                # Raw buffers per slice, viewed per pipeline stage:
                # TZ f32: z input -> oi output; TA f32: atbT input;
                # TR bf16: rhs (p1 stat) -> ar2 (p3 out / p4 stat);
                # TB bf16: ar (p1 out / p2 stat) -> gr (chi-mult out / p3 stat);
                # TH bf16: hr (p2 out).
                TZ, TA, TR, TB, TH = [], [], [], [], []
                for b in range(2):
                    TZ.append(dpool.tile([P, 4096], mybir.dt.float32, tag="tz",
                                         name=f"tz{b}"))
                    TA.append(dpool.tile([P, 4096], mybir.dt.float32, tag="ta",
                                         name=f"ta{b}"))
                    TR.append(dpool.tile([P, 4096], dt_bf, tag="tr",
                                         name=f"tr{b}"))
                    TB.append(dpool.tile([P, 4096], dt_bf, tag="tb",
                                         name=f"tb{b}"))
                    TH.append(dpool.tile([P, 4096], dt_bf, tag="th",
                                         name=f"th{b}"))

                def v_bwk(t):   # [P, JT, W, 2]
                    return t[:].rearrange("p (jt w k) -> p jt w k",
                                          jt=JT, w=W, k=2)

                def v_bkw(t):   # [P, JT, 2, W]
                    return t[:].rearrange("p (jt k w) -> p jt k w",
                                          jt=JT, k=2, w=W)

                cht = cpool.tile([P, JT, W], dt_bf, tag="chi", name="cht")

                src = "b (sub p par) c k -> b p par sub c k"
                v = "p (par sub) c k -> p par sub c k"
                zsr = zs.rearrange(src, sub=2, p=P, par=2)
                asr = as_.rearrange(src, sub=2, p=P, par=2)
                chv = chi.rearrange("jt p c -> p jt c")

                G3f = G3c = None
                for b, cc in ((0, 0), (0, 1), (1, 0), (1, 1)):
                    cs = slice(cc * 256, (cc + 1) * 256)
                    zv = v_bwk(TZ[b]).rearrange(v, par=2, sub=2)
                    av = v_bwk(TA[b]).rearrange(v, par=2, sub=2)
                    nc.sync.dma_start(zv[:, :, :, cs, :], zsr[b][:, :, :, cs, :])
                    nc.sync.dma_start(av[:, :, :, cs, :], asr[b][:, :, :, cs, :])
                    if b == 0 and cc == 0:
                        G3f = load_consts(nc, cpool, gaps_f, fwd_names)
                    if b == 1 and cc == 0:
                        G3c = load_consts(nc, cpool, gaps_c, con_names)
                        for q in range(4):
                            nc.sync.dma_start(cht[:, q, :], chv[:, q, :])
                warmup(nc, cpool, psp)

                # rhs = atbT + lam*z on scalar+gpsimd (keep DVE free), in
                # column halves (cc0 first so pass 1's q in {0,2} can start
                # before the cc1 DMAs land)
                for cc in range(2):
                    cs = slice(cc * 256, (cc + 1) * 256)
                    for b in range(2):
                        zt, at = v_bwk(TZ[b]), v_bwk(TA[b])
                        rt = v_bwk(TR[b])
                        nc.scalar.mul(zt[:, :, cs, :], zt[:, :, cs, :], LAM)
                        nc.gpsimd.tensor_add(rt[:, :, cs, :], at[:, :, cs, :],
                                             zt[:, :, cs, :])

                def strided(plane_kwc, jt, q, comp):
                    # [P, 128] stationary: stride-2 column block of quarter q
                    start = 256 * (q % 2) + q // 2
                    return plane_kwc[:, jt, start:start + 255:2, comp]

                def strided_kw(plane_kw, jt, q, comp):
                    start = 256 * (q % 2) + q // 2
                    return plane_kw[:, jt, comp, start:start + 255:2]

                # pass 1 (fwd): rt -> ar (TB)
                for b in range(2):
                    rt, ar = v_bwk(TR[b]), v_bkw(TB[b])

                    def stat1(jt, q, comp, rt=rt):
                        return strided(rt, jt, q, comp)

                    def emit_a(q, ps_e, t_sb, ar=ar):
                        comb(nc, ar, q, ps_e, t_sb)

                    dft_pass(nc, psp, dpool, stat1, G3f, emit_a,
                             qs=(0, 2, 1, 3), tsb_dt=dt_bf)

                # pass 2 (fwd): ar -> hr (TH), then gr = chi*hr into TB
                for b in range(2):
                    ar, hr = v_bkw(TB[b]), v_bkw(TH[b])

                    def stat2(jt, q, comp, ar=ar):
                        return strided_kw(ar, jt, q, comp)

                    def emit_h(q, ps_e, t_sb, hr=hr):
                        comb(nc, hr, q, ps_e, t_sb)

                    dft_pass(nc, psp, dpool, stat2, G3f, emit_h, tsb_dt=dt_bf)

                for b in range(2):
                    hr, gr = v_bkw(TH[b]), v_bkw(TB[b])
                    for q in range(4):
                        nc.vector.tensor_mul(gr[:, q, 0, :], hr[:, q, 0, :],
                                             cht[:, q, :])
                        nc.gpsimd.tensor_mul(gr[:, q, 1, :], hr[:, q, 1, :],
                                             cht[:, q, :])

                # pass 3 (conj): gr -> ar2 (TR)
                for b in range(2):
                    gr, ar2 = v_bkw(TB[b]), v_bkw(TR[b])

                    def stat3(jt, q, comp, gr=gr):
                        return strided_kw(gr, jt, q, comp)

                    def emit_a2(q, ps_e, t_sb, ar2=ar2):
                        comb(nc, ar2, q, ps_e, t_sb)

                    dft_pass(nc, psp, dpool, stat3, G3c, emit_a2, tsb_dt=dt_bf)

                # pass 4 (conj): ar2 -> oi (TZ), DMA out per q
                for b in range(2):
                    ar2, oi = v_bkw(TR[b]), v_bwk(TZ[b])

                    def stat4(jt, q, comp, ar2=ar2):
                        return strided_kw(ar2, jt, q, comp)

                    def emit_o(q, ps_e, t_sb, b=b, oi=oi):
                        e2 = ps_e[:].rearrange("p (k c) -> p k c", k=2)
                        t2 = t_sb[:].rearrange("p (k c) -> p k c", k=2)
                        lo = oi[:, q, 0:256, :].rearrange("p c k -> p k c")
                        hi = oi[:, q, 256:512, :].rearrange("p c k -> p k c")
                        nc.vector.tensor_add(lo, e2, t2)
                        nc.vector.tensor_sub(hi, e2, t2)
                        dstp = "b (sub p par) c k -> b p par sub c k"
                        ov = out.rearrange(dstp, sub=2, p=P, par=2)[b]
                        nc.sync.dma_start(ov[:, q // 2, q % 2], oi[:, q])

                    dft_pass(nc, psp, dpool, stat4, G3c, emit_o,
                             tsb_dt=mybir.dt.float32)

        nc.compile()
        return nc

    return build()


LAST_EXEC_NS = {}


def kernel(z, atbT, mask):
    import os
    import ml_dtypes
    from concourse.bass_utils import run_bass_kernel_spmd

    trace = bool(os.environ.get("DC_TRACE"))

    if "k" not in _cache:
        _cache["k"] = _build_kernel()
    nck = _cache["k"]

    Gf = dict(zip(["a1", "a2", "t1", "t2"], _make_consts(conj=False)))
    Gc = dict(zip(["c1", "c2", "c3", "c4"], _make_consts(conj=True)))
    perm = _perm_rows()

    z = np.ascontiguousarray(np.asarray(z, dtype=np.float32))
    atbT = np.ascontiguousarray(np.asarray(atbT, dtype=np.float32))
    mask = np.asarray(mask, dtype=np.float32)

    # Uniform-weight collapsed CG filter from mask alone (host, free).
    d_dev = (mask.astype(np.float64) + LAM)[perm]
    chi_dev = _collapsed_cg(d_dev, np.ones_like(d_dev)) / (512.0 * 512.0)
    chi_t = np.ascontiguousarray(
        chi_dev.astype(np.float32).astype(ml_dtypes.bfloat16).reshape(JT, P, W))

    in_maps = [
        {"zs": np.ascontiguousarray(z[2 * c:2 * c + 2]),
         "as_": np.ascontiguousarray(atbT[2 * c:2 * c + 2]),
         "chi": chi_t, **Gf, **Gc}
        for c in range(N_CORES)
    ]
    res = run_bass_kernel_spmd(nck, in_maps, core_ids=list(range(N_CORES)),
                               trace=trace)
    if trace:
        LAST_EXEC_NS["a"] = res.exec_time_ns

    return np.concatenate([res.results[c]["out"] for c in range(N_CORES)], axis=0)
